# revision 1
# baseline (speedup 1.0000x reference)
"""Euclidean distance block (retrieval kNN) on 8 TRN2 NeuronCores.

dist[b, s, p] = sqrt(sum_c (x1[b, c, p] - x2[b, s, c, p])^2)   p = spatial (h*w)
out[b] = dist[b].reshape(S * h * w)

Sharding: data-parallel over batch B=32 -> 4 batches per core, no comms.

Per-core kernel layout: SBUF partitions carry (support_pair, channel) = 2*64 =
128; the free axis carries spatial. A big tile covers 8 supports as
[128, 4, 1764], streamed as four fully-contiguous 902 KB pair-DMAs (f32 HBM
-> bf16 SBUF cast on the SWDGE ring; per-pair DMAs give 4x finer completion
sems so compute starts on the first pair). Compute chain per tile:
  DVE subtract in bf16 (2x mode), in place
  Square -> bf16: 3 slices on ACT, 1 on DVE (engine cadence balance)
  PE matmul against [128, 25] one-hot pair masks, accumulating per-support
    sums over C into a [25, 441] PSUM tile per spatial quarter
  ACT Sqrt PSUM -> SBUF f32, one contiguous 176 KB store per batch on the
    Scalar HWDGE ring (which never blocks loads).

DMA ring assignment matters: HWDGE rings execute FIFO per issuing engine, so
a store waiting on compute would stall every load queued behind it. Loads
(with cast) go on the GpSimd SWDGE ring, stores on Scalar, and the sync ring
only carries the mask load and the x1 partition-duplicate copy.
"""

import numpy as np

B, S, C, H, W = 32, 25, 64, 42, 42
HW = H * W            # 1764
NCORES = 8
BL = B // NCORES      # 4 batches per core
NSO = 4               # support pairs per big tile (8 supports)
NBIG = 3              # big tiles per batch (24 supports), then 1 leftover
NQ = 4                # spatial quarters
QW = HW // NQ         # 441
NPAIR = 13            # 12 support pairs + 1 leftover single

BF16_SUB = True       # bf16 inputs to the subtract (2x DVE); False = f32
F32_RAMP = False      # stream b0's first tile in f32 on the sync ring

_cache = {}


def _build_nc():
    import concourse.bacc as bacc
    import concourse.mybir as mybir
    from concourse.tile import TileContext
    from concourse.bass import MemorySpace

    f32 = mybir.dt.float32
    bf16 = mybir.dt.bfloat16
    ldt = bf16 if BF16_SUB else f32
    Square = mybir.ActivationFunctionType.Square
    Sqrt = mybir.ActivationFunctionType.Sqrt
    sub = mybir.AluOpType.subtract

    # Square and Sqrt both live in the "sqrt_and_others" act-function set,
    # but the table-load chooser picks the first set containing each one,
    # alternating two ~2.7us table reloads per batch. Strip the two
    # functions from every other set (contents only — set ids are
    # positional) so one resident table serves the whole kernel.
    _orig_tables = bacc.get_activation_tables

    def _pinned_tables(arch):
        t = _orig_tables(arch)
        for name, fns in t.items():
            if name != "sqrt_and_others":
                fns.discard(Square)
                fns.discard(Sqrt)
        return t

    bacc.get_activation_tables = _pinned_tables
    nc = bacc.Bacc()
    x1 = nc.declare_dram_parameter("x1", [BL, C, HW], f32, isOutput=False)
    x2 = nc.declare_dram_parameter("x2", [BL, S, C, HW], f32, isOutput=False)
    mk = nc.declare_dram_parameter("mask", [NPAIR, 128, S], bf16, isOutput=False)
    out = nc.declare_dram_parameter("out", [BL, S * HW], f32, isOutput=True)

    # loads: cast f32->ldt needs SWDGE (gpsimd); plain copies can go anywhere
    load = nc.gpsimd if BF16_SUB else nc.sync

    with TileContext(nc) as tc:
        with (
            tc.tile_pool(name="x2p", bufs=6) as x2p,
            tc.tile_pool(name="sqp", bufs=3) as sqp,
            tc.tile_pool(name="x1p", bufs=1) as x1p,
            tc.tile_pool(name="outp", bufs=2) as outp,
            tc.tile_pool(name="cst", bufs=1) as cst,
            tc.tile_pool(name="x2fp", bufs=1) as x2fp,
            tc.tile_pool(name="ps", bufs=2, space=MemorySpace.PSUM) as psp,
        ):
            mt = cst.tile([128, NPAIR, S], bf16)
            nc.sync.dma_start(mt[:], mk.rearrange("g k m -> k g m"))

            # all of x1 once: [c, b, p] on partitions 0..63, then duplicate
            # onto 64..127 via SBUF->SBUF (no extra HBM traffic)
            x1all = x1p.tile([128, BL, HW], ldt)
            load.dma_start(x1all[0:64, :, :], x1.rearrange("b c p -> c b p"))
            nc.sync.dma_start(x1all[64:128, :, :], x1all[0:64, :, :])

            # The first gpsimd (SWDGE) DMA pays ~6us of Q7 library-load +
            # descriptor warmup before the first HBM byte moves. Stream the
            # first batch's first tile (and its x1) in f32 over the sync
            # HWDGE ring instead, so HBM traffic starts immediately.
            x1f = None
            if BF16_SUB and F32_RAMP:
                x1f = cst.tile([128, HW], f32, name="x1f")
                nc.sync.dma_start(x1f[0:64, :], x1[0])
                nc.sync.dma_start(x1f[64:128, :], x1[0])

            for b in range(BL):
                # b=0 starts on the warm sync ring in f32; leftover-first
                # ordering only for b>0 (for b=0 the leftover data arrives
                # late, after the gpsimd warmup)
                leftover_first = b > 0 or not (BF16_SUB and F32_RAMP)

                # leftover support 24: DMA early so it streams with big tiles
                x2l = x2p.tile([64, HW], ldt, tag="x2l")
                load.dma_start(x2l[:], x2[b, S - 1])

                pst = [
                    psp.tile([S, QW], f32, name=f"ps{q}", tag=f"ps{q}")
                    for q in range(NQ)
                ]

                def leftover_compute(b=b, x2l=x2l, pst=pst, first=True):
                    # leftover compute first keeps the end-of-batch tail short
                    nc.vector.tensor_tensor(x2l[:], x2l[:], x1all[0:64, b, :], sub)
                    sql = sqp.tile([64, HW], bf16, name="sql", tag="sql")
                    nc.scalar.activation(sql[:], x2l[:], Square)
                    for q in range(NQ):
                        nc.tensor.matmul(
                            pst[q][:, :],
                            mt[0:64, NPAIR - 1, :],
                            sql[:, q * QW : (q + 1) * QW],
                            start=first,
                            stop=not first,
                        )

                if leftover_first:
                    leftover_compute(first=True)

                for i in range(NBIG):
                    f32_tile = BF16_SUB and F32_RAMP and b == 0 and i == 0
                    if f32_tile:
                        x2t = x2fp.tile([128, NSO, HW], f32, name="x2tf", tag="x2tf")
                        x1s = x1f[:, :]
                        ring = nc.sync
                    else:
                        x2t = x2p.tile([128, NSO, HW], ldt, tag="x2t")
                        x1s = x1all[:, b, :]
                        ring = load
                    src = x2[b, 8 * i : 8 * i + 8].rearrange(
                        "(so si) c p -> (si c) so p", si=2
                    )
                    # per-pair DMAs: same streaming rate, but 4x finer
                    # completion sems -> subs start on the first 902KB
                    for so in range(NSO):
                        ring.dma_start(x2t[:, so, :], src[:, so, :])
                    # The very last tile's chain is the kernel tail: q-slice
                    # its compute so the final dependency chain is one
                    # 441-wide chunk instead of a whole 1764-wide slice.
                    last_tile = b == BL - 1 and i == NBIG - 1
                    ot = None
                    if last_tile:
                        ot = outp.tile([S, HW], f32, name="ot", tag="ot")
                    sq = sqp.tile([128, NSO, HW], bf16, tag="sq")
                    for so in range(NSO):
                        j = NSO * i + so
                        if not last_tile:
                            # in-place: x2t slice becomes diff
                            nc.vector.tensor_tensor(
                                x2t[:, so, :], x2t[:, so, :], x1s, sub
                            )
                            # squares split 3/1 across ACT and DVE to balance
                            # the per-tile engine cadence
                            if so < 3:
                                nc.scalar.activation(
                                    sq[:, so, :], x2t[:, so, :], Square
                                )
                            else:
                                nc.vector.tensor_tensor(
                                    sq[:, so, :],
                                    x2t[:, so, :],
                                    x2t[:, so, :],
                                    mybir.AluOpType.mult,
                                )
                            for q in range(NQ):
                                nc.tensor.matmul(
                                    pst[q][:, :],
                                    mt[:, j, :],
                                    sq[:, so, q * QW : (q + 1) * QW],
                                    start=(j == 0 and not leftover_first),
                                    stop=(j == NPAIR - 2 and leftover_first),
                                )
                        else:
                            for q in range(NQ):
                                qs = slice(q * QW, (q + 1) * QW)
                                nc.vector.tensor_tensor(
                                    x2t[:, so, qs], x2t[:, so, qs], x1s[:, qs], sub
                                )
                                if q % 2 == 0:
                                    nc.scalar.activation(
                                        sq[:, so, qs], x2t[:, so, qs], Square
                                    )
                                else:
                                    nc.vector.tensor_tensor(
                                        sq[:, so, qs],
                                        x2t[:, so, qs],
                                        x2t[:, so, qs],
                                        mybir.AluOpType.mult,
                                    )
                                nc.tensor.matmul(
                                    pst[q][:, :],
                                    mt[:, j, :],
                                    sq[:, so, qs],
                                    start=(j == 0 and not leftover_first),
                                    stop=(j == NPAIR - 2 and leftover_first),
                                )
                                if so == NSO - 1:
                                    # quarter q is complete: sqrt + store now
                                    nc.scalar.activation(
                                        ot[:, qs], pst[q][:], Sqrt
                                    )
                                    nc.scalar.dma_start(
                                        out[b].rearrange("(s p) -> s p", s=S)[:, qs],
                                        ot[:, qs],
                                    )

                if not leftover_first:
                    leftover_compute(first=False)

                if b < BL - 1:
                    ot = outp.tile([S, HW], f32, name="ot", tag="ot")
                    for q in range(NQ):
                        nc.scalar.activation(
                            ot[:, q * QW : (q + 1) * QW], pst[q][:], Sqrt
                        )
                    # store via the Scalar HWDGE ring: ACT reaches this only
                    # after its own sqrts, so the wait is pre-satisfied; a
                    # store on a load ring would stall loads queued behind it
                    nc.scalar.dma_start(out[b].rearrange("(s p) -> s p", s=S), ot[:])

    try:
        nc.finalize()
    finally:
        bacc.get_activation_tables = _orig_tables
    return nc


def get_nc():
    if "nc" not in _cache:
        _cache["nc"] = _build_nc()
    return _cache["nc"]


def make_mask() -> np.ndarray:
    # mask[j, k, m] = 1 iff partition k of pair-tile j feeds output support m.
    # Pair j < 12 covers supports (2j, 2j+1): k < 64 -> 2j, k >= 64 -> 2j+1.
    # Pair 12 is the leftover single support 24 on partitions 0..63.
    import ml_dtypes

    mask = np.zeros((NPAIR, 128, S), dtype=ml_dtypes.bfloat16)
    for j in range(NPAIR - 1):
        mask[j, 0:64, 2 * j] = 1.0
        mask[j, 64:128, 2 * j + 1] = 1.0
    mask[NPAIR - 1, 0:64, S - 1] = 1.0
    return mask


def make_in_maps(x1: np.ndarray, x2: np.ndarray) -> list[dict]:
    x1 = np.ascontiguousarray(np.asarray(x1, dtype=np.float32)).reshape(B, C, HW)
    x2 = np.ascontiguousarray(np.asarray(x2, dtype=np.float32)).reshape(B, S, C, HW)
    mask = make_mask()
    maps = []
    for i in range(NCORES):
        sl = slice(i * BL, (i + 1) * BL)
        maps.append({"x1": x1[sl], "x2": x2[sl], "mask": mask})
    return maps


def gather_out(results: list[dict]) -> np.ndarray:
    return np.concatenate([np.asarray(r["out"]) for r in results], axis=0).astype(
        np.float32, copy=False
    )


def kernel(x1, x2) -> np.ndarray:
    from concourse.bass_utils import run_bass_kernel_spmd

    nc = get_nc()
    in_maps = make_in_maps(x1, x2)
    res = run_bass_kernel_spmd(nc, in_maps, list(range(NCORES)))
    return gather_out(res.results)



# revision 4
# speedup vs baseline: 1.4174x; 1.4174x over previous
"""Euclidean distance block (retrieval kNN) on 8 TRN2 NeuronCores.

dist[b, s, p] = sqrt(sum_c (x1[b, c, p] - x2[b, s, c, p])^2)   p = spatial (h*w)
out[b] = dist[b].reshape(S * h * w)

Sharding: data-parallel over batch B=32 -> 4 batches per core, no comms.

v2 design (evolved from the f32/SWDGE baseline at ~145-166us traced):

1. HOST-SIDE bf16 STAGING. The baseline streamed x2 as f32 (45 MB/core) and
   cast f32->bf16 on the SWDGE ring; the subtract was already bf16, so
   pre-casting x2/x1 to bf16 on the host gives identical numerics with HALF
   the HBM read traffic (22.6 MB/core) and removes the cast -> every load is
   a plain HWDGE DMA (sync ring, ~0.6us first byte, no ~6us Q7 warmup).
   x1 is also pre-duplicated on host to [128=(2x64c), BL, HW] so the kernel
   needs no SBUF->SBUF partition duplicate. Output is stored bf16 and
   upcast to f32 on host (rel err budget 2e-2, bf16 adds <0.4%).

2. PAIR-GRANULAR PIPELINE. SBUF partitions carry (support_pair, channel) =
   2*64 = 128. Each support pair (si c) is one 451KB HWDGE load, then:
   DVE subtract in place (bf16 2x mode), square -> sq tile (split over
   ACT/DVE/GpSimd to balance engine load; GpSimd is free now that SWDGE is
   gone), PE mask-matmul accumulating per-support sums over C into
   [25, 441] PSUM tiles (4 spatial quarters), ACT sqrt -> bf16 store on the
   scalar HWDGE ring (loads and stores never share a FIFO).

3. PE KEEP-WARM FILLERS. TRN2's power manager runs the PE at HALF clock
   (371ns per 441-col matmul) unless it has been continuously busy for
   ~3.4us, full clock (188ns) after. Per-pair bursts (~1.9us) with gaps
   never promote. Zero-weight accumulate matmuls (mask column 13 is all
   zeros -> adds 0.0 to live PSUM, numerically a no-op) pad the gaps so the
   PE stays busy and promoted.

4. SHORT TAIL. The last batch computes the half-width leftover support 24
   LAST, quarter-sliced: its 4x56KB loads are the final DMAs and each
   quarter's sub->square->matmul(stop)->sqrt->store chain fires as its
   56KB lands, so the post-last-byte critical path is one 441-wide chain.
"""

import numpy as np

B, S, C, H, W = 32, 25, 64, 42, 42
HW = H * W            # 1764
NCORES = 8
BL = B // NCORES      # 4 batches per core
NPAIR = 12            # full support pairs (24 supports); support 24 leftover
NQ = 4                # spatial quarters
QW = HW // NQ         # 441
NMASK = 14            # 12 pair masks + leftover mask (12) + zero filler (13)
LO = 12               # mask index of the leftover support
ZW = 13               # mask index of the all-zero filler weights

NFILL = 2             # keep-warm fillers per pair
FILLW = 441           # filler matmul moving columns

# square-engine schedule per pair j (A=ACT, D=DVE mult, G=GpSimd mult)
SQ_ENG = "ADAGAADAGAAD"

_cache = {}


def _build_nc():
    import concourse.bacc as bacc
    import concourse.mybir as mybir
    from concourse.tile import TileContext
    from concourse.bass import MemorySpace

    f32 = mybir.dt.float32
    bf16 = mybir.dt.bfloat16
    Square = mybir.ActivationFunctionType.Square
    Sqrt = mybir.ActivationFunctionType.Sqrt
    sub = mybir.AluOpType.subtract
    mul = mybir.AluOpType.mult

    # Square and Sqrt both live in the "sqrt_and_others" act-function set,
    # but the table-load chooser picks the first set containing each one,
    # alternating two ~2.7us table reloads per batch. Strip the two
    # functions from every other set (contents only - set ids are
    # positional) so one resident table serves the whole kernel.
    _orig_tables = bacc.get_activation_tables

    def _pinned_tables(arch):
        t = _orig_tables(arch)
        for name, fns in t.items():
            if name != "sqrt_and_others":
                fns.discard(Square)
                fns.discard(Sqrt)
        return t

    bacc.get_activation_tables = _pinned_tables
    nc = bacc.Bacc()
    x1 = nc.declare_dram_parameter("x1", [128, BL, HW], bf16, isOutput=False)
    x2 = nc.declare_dram_parameter("x2", [BL, S, C, HW], bf16, isOutput=False)
    mk = nc.declare_dram_parameter("mask", [NMASK, 128, S], bf16, isOutput=False)
    out = nc.declare_dram_parameter("out", [BL, S * HW], bf16, isOutput=True)

    with TileContext(nc) as tc:
        with (
            tc.tile_pool(name="x2p", bufs=20) as x2p,
            tc.tile_pool(name="lop", bufs=2) as lop,
            tc.tile_pool(name="sqp", bufs=8) as sqp,
            tc.tile_pool(name="sqlp", bufs=2) as sqlp,
            tc.tile_pool(name="x1p", bufs=1) as x1p,
            tc.tile_pool(name="outp", bufs=2) as outp,
            tc.tile_pool(name="cst", bufs=1) as cst,
            tc.tile_pool(name="ps", bufs=2, space=MemorySpace.PSUM) as psp,
        ):
            mt = cst.tile([128, NMASK, S], bf16)
            nc.sync.dma_start(mt[:], mk.rearrange("g k m -> k g m"))

            x1t = x1p.tile([128, BL, HW], bf16)
            nc.sync.dma_start(x1t[:, 0, :], x1[:, 0, :])

            def filler(pst, j, sq, n=NFILL):
                # zero-weight accumulates: keep the PE busy through the
                # per-pair DMA gap so the power manager holds full clock
                for k in range(n):
                    nc.tensor.matmul(
                        pst[(j + k) % NQ][:, :],
                        mt[:, ZW, :],
                        sq[:, :FILLW],
                        start=False,
                        stop=False,
                        skip_group_check=True,
                    )

            for b in range(BL):
                last = b == BL - 1

                pst = [
                    psp.tile([S, QW], f32, name=f"ps{q}", tag=f"ps{q}")
                    for q in range(NQ)
                ]

                if not last:
                    # leftover support 24 first: its sqrt/store tail then
                    # overlaps the next batch's stream
                    x2l = lop.tile([64, HW], bf16, tag="lo")
                    nc.sync.dma_start(x2l[:], x2[b, S - 1])

                # per-pair loads; batch b+1's x1 slice rides after pair 0
                pairs = []
                for j in range(NPAIR):
                    x2t = x2p.tile([128, HW], bf16, tag="x2t")
                    nc.sync.dma_start(
                        x2t[:], x2[b, 2 * j : 2 * j + 2].rearrange("si c p -> (si c) p")
                    )
                    pairs.append(x2t)
                    if j == 0 and b + 1 < BL:
                        nc.sync.dma_start(x1t[:, b + 1, :], x1[:, b + 1, :])
                if last:
                    # leftover is the kernel tail: quarter-sliced, loaded last
                    x2l = lop.tile([64, HW], bf16, tag="lo")
                    for q in range(NQ):
                        nc.sync.dma_start(
                            x2l[:, q * QW : (q + 1) * QW],
                            x2[b, S - 1][:, q * QW : (q + 1) * QW],
                        )

                if not last:
                    nc.vector.tensor_tensor(x2l[:], x2l[:], x1t[0:64, b, :], sub)
                    sql = sqlp.tile([64, HW], bf16, name="sql", tag="sql")
                    nc.scalar.activation(sql[:], x2l[:], Square)
                    for q in range(NQ):
                        nc.tensor.matmul(
                            pst[q][:, :],
                            mt[0:64, LO, :],
                            sql[:, q * QW : (q + 1) * QW],
                            start=True,
                            stop=False,
                        )

                for j in range(NPAIR):
                    x2t = pairs[j]
                    nc.vector.tensor_tensor(x2t[:], x2t[:], x1t[:, b, :], sub)
                    sq = sqp.tile([128, HW], bf16, tag="sq")
                    eng = SQ_ENG[j]
                    if eng == "A":
                        nc.scalar.activation(sq[:], x2t[:], Square)
                    elif eng == "D":
                        nc.vector.tensor_tensor(sq[:], x2t[:], x2t[:], mul)
                    else:
                        nc.gpsimd.tensor_tensor(sq[:], x2t[:], x2t[:], mul)
                    for q in range(NQ):
                        nc.tensor.matmul(
                            pst[q][:, :],
                            mt[:, j, :],
                            sq[:, q * QW : (q + 1) * QW],
                            start=(last and j == 0),
                            stop=(not last and j == NPAIR - 1),
                        )
                    filler(pst, j, sq)

                if not last:
                    ot = outp.tile([S, HW], bf16, name="ot", tag="ot")
                    for q in range(NQ):
                        nc.scalar.activation(
                            ot[:, q * QW : (q + 1) * QW], pst[q][:], Sqrt
                        )
                    nc.scalar.dma_start(out[b].rearrange("(s p) -> s p", s=S), ot[:])
                else:
                    # tail: leftover quarters stream in as the final DMAs;
                    # each quarter's chain fires on its own 56KB completion
                    ot = outp.tile([S, HW], bf16, name="ot", tag="ot")
                    sql = sqlp.tile([64, HW], bf16, name="sql", tag="sql")
                    for q in range(NQ):
                        qs = slice(q * QW, (q + 1) * QW)
                        nc.vector.tensor_tensor(
                            x2l[:, qs], x2l[:, qs], x1t[0:64, b, qs], sub
                        )
                        nc.scalar.activation(sql[:, qs], x2l[:, qs], Square)
                        nc.tensor.matmul(
                            pst[q][:, :],
                            mt[0:64, LO, :],
                            sql[:, qs],
                            start=False,
                            stop=True,
                        )
                        nc.scalar.activation(ot[:, qs], pst[q][:], Sqrt)
                        nc.scalar.dma_start(
                            out[b].rearrange("(s p) -> s p", s=S)[:, qs], ot[:, qs]
                        )

    try:
        nc.finalize()
    finally:
        bacc.get_activation_tables = _orig_tables
    return nc


def get_nc():
    if "nc" not in _cache:
        _cache["nc"] = _build_nc()
    return _cache["nc"]


def make_mask() -> np.ndarray:
    # mask[j, k, m] = 1 iff partition k of pair-tile j feeds output support m.
    # Pair j < 12 covers supports (2j, 2j+1): k < 64 -> 2j, k >= 64 -> 2j+1.
    # Slot 12 is the leftover single support 24 on partitions 0..63.
    # Slot 13 is all zeros: weights for the PE keep-warm filler matmuls.
    import ml_dtypes

    mask = np.zeros((NMASK, 128, S), dtype=ml_dtypes.bfloat16)
    for j in range(NPAIR):
        mask[j, 0:64, 2 * j] = 1.0
        mask[j, 64:128, 2 * j + 1] = 1.0
    mask[LO, 0:64, S - 1] = 1.0
    return mask


def make_in_maps(x1: np.ndarray, x2: np.ndarray) -> list[dict]:
    import ml_dtypes

    bf16 = ml_dtypes.bfloat16
    x1 = np.asarray(x1, dtype=np.float32).reshape(B, C, HW)
    x2 = np.asarray(x2, dtype=np.float32).reshape(B, S, C, HW)
    mask = make_mask()
    maps = []
    for i in range(NCORES):
        sl = slice(i * BL, (i + 1) * BL)
        # x1 staged bf16, channel-major, duplicated onto both partition
        # halves so it aligns with the (si c) pair layout
        x1c = np.ascontiguousarray(x1[sl].transpose(1, 0, 2)).astype(bf16)
        x1d = np.ascontiguousarray(np.concatenate([x1c, x1c], axis=0))
        x2c = np.ascontiguousarray(x2[sl]).astype(bf16)
        maps.append({"x1": x1d, "x2": x2c, "mask": mask})
    return maps


def gather_out(results: list[dict]) -> np.ndarray:
    return np.concatenate([np.asarray(r["out"]) for r in results], axis=0).astype(
        np.float32
    )


def kernel(x1, x2) -> np.ndarray:
    from concourse.bass_utils import run_bass_kernel_spmd

    nc = get_nc()
    in_maps = make_in_maps(x1, x2)
    res = run_bass_kernel_spmd(nc, in_maps, list(range(NCORES)))
    return gather_out(res.results)


# revision 7
# speedup vs baseline: 1.7010x; 1.2000x over previous
"""Euclidean distance block (retrieval kNN) on 8 TRN2 NeuronCores.

dist[b, s, p] = sqrt(sum_c (x1[b, c, p] - x2[b, s, c, p])^2)   p = spatial (h*w)
out[b] = dist[b].reshape(S * h * w)

Sharding: data-parallel over batch B=32 -> 4 batches per core, no comms.

Design (v3; baseline f32/SWDGE was ~145-166us traced, v2 ~120us):

1. HOST-SIDE bf16 STAGING. The baseline streamed x2 as f32 (45 MB/core) and
   cast f32->bf16 on the SWDGE ring; the subtract was already bf16, so
   pre-casting x2/x1 to bf16 on the host gives identical numerics with HALF
   the HBM read traffic (22.6 MB/core) and removes the cast -> every load is
   a plain HWDGE DMA (sync ring, ~0.6us first byte, no ~6us Q7 warmup).
   x1 is also pre-duplicated on host to [128=(2x64c), BL, HW] so the kernel
   needs no SBUF->SBUF partition duplicate. Output is stored bf16 and
   upcast to f32 on host (rel err budget 2e-2, bf16 adds <0.4%).

2. PAIR-GRANULAR PIPELINE. SBUF partitions carry (support_pair, channel) =
   2*64 = 128. Each support pair (si c) is one 451KB HWDGE load, then:
   DVE subtract in place (bf16 2x mode), square -> sq tile (8 on ACT, 4 on
   DVE per batch; GpSimd tensor ops measured 5x slow - never use), PE
   mask-matmul accumulating per-support sums over C into [25, 441] PSUM
   tiles (4 spatial quarters), ACT sqrt -> bf16 store on the scalar HWDGE
   ring (loads and stores never share a FIFO).

3. PE KEEP-WARM FILLERS. TRN2's power manager runs the PE at HALF clock
   (371ns per 441-col matmul) unless it has been continuously busy for
   ~3.4us, full clock (188ns) after. Per-pair bursts (~1.5us) with gaps
   never promote. Fillers = matmuls of a zeroed SBUF tile with whatever
   weights are resident (ldweights=False) accumulated into live PSUM:
   adds 0.0, costs no weight reload, keeps the PE promoted.

4. LDWEIGHTS ELISION. The 4 quarter-matmuls of a pair share one mask; only
   quarter 0 self-loads weights (ldweights=False on the rest elides the
   ~101ns InstLdweights each). _verify_ldw_order() walks the final BIR and
   asserts no foreign weight load lands between a loader and its dependents
   (the Tile scheduler could in principle reorder same-engine matmuls).

5. SHORT TAIL. The last batch computes the half-width leftover support 24
   LAST, quarter-sliced: its 4x56KB loads are the final DMAs and each
   quarter's sub->square->matmul(stop)->sqrt->store chain fires as its
   56KB lands, so the post-last-byte critical path is one 441-wide chain.
"""

import numpy as np

B, S, C, H, W = 32, 25, 64, 42, 42
HW = H * W            # 1764
NCORES = 8
BL = B // NCORES      # 4 batches per core
NPAIR = 12            # full support pairs (24 supports); support 24 leftover
NQ = 4                # spatial quarters
QW = HW // NQ         # 441
NMASK = 14            # 12 pair masks + leftover mask (12) + zero filler (13)
LO = 12               # mask index of the leftover support
ZW = 13               # mask index of the all-zero filler weights

NFILL = 2             # keep-warm fillers per pair
FILLW = 441           # filler matmul moving columns
ELIDE_LDW = False     # legalization re-pairs an InstLdweights with every
                      # matmult regardless; LDW overlaps MM execution anyway

# square-engine schedule per pair j (A=ACT, D=DVE mult)
SQ_ENG = "AADAADAADAAD"

_cache = {}


def _build_nc():
    import concourse.bacc as bacc
    import concourse.mybir as mybir
    from concourse.tile import TileContext
    from concourse.bass import MemorySpace

    f32 = mybir.dt.float32
    bf16 = mybir.dt.bfloat16
    Square = mybir.ActivationFunctionType.Square
    Sqrt = mybir.ActivationFunctionType.Sqrt
    sub = mybir.AluOpType.subtract
    mul = mybir.AluOpType.mult

    # Square and Sqrt both live in the "sqrt_and_others" act-function set,
    # but the table-load chooser picks the first set containing each one,
    # alternating two ~2.7us table reloads per batch. Strip the two
    # functions from every other set (contents only - set ids are
    # positional) so one resident table serves the whole kernel.
    _orig_tables = bacc.get_activation_tables

    def _pinned_tables(arch):
        t = _orig_tables(arch)
        for name, fns in t.items():
            if name != "sqrt_and_others":
                fns.discard(Square)
                fns.discard(Sqrt)
        return t

    bacc.get_activation_tables = _pinned_tables
    nc = bacc.Bacc()
    x1 = nc.declare_dram_parameter("x1", [128, BL, HW], bf16, isOutput=False)
    x2 = nc.declare_dram_parameter("x2", [BL, S, C, HW], bf16, isOutput=False)
    mk = nc.declare_dram_parameter("mask", [NMASK, 128, S], bf16, isOutput=False)
    out = nc.declare_dram_parameter("out", [BL, S * HW], bf16, isOutput=True)

    # build-time bookkeeping for _verify_ldw_order
    elide_owner = {}      # elided matmult name -> its weight-loader's name
    filler_names = set()

    def mm(pst_q, w, mov, start, stop, loader=None, skip=False):
        inst = nc.tensor.matmul(
            pst_q, w, mov, start=start, stop=stop, skip_group_check=skip
        )
        if loader is not None and ELIDE_LDW:
            inst.ins.ldweights = False
            elide_owner[inst.ins.name] = loader.ins.name
        return inst

    with TileContext(nc) as tc:
        with (
            tc.tile_pool(name="x2p", bufs=12) as x2p,
            tc.tile_pool(name="lop", bufs=2) as lop,
            tc.tile_pool(name="sqp", bufs=6) as sqp,
            tc.tile_pool(name="sqlp", bufs=2) as sqlp,
            tc.tile_pool(name="x1p", bufs=1) as x1p,
            tc.tile_pool(name="outp", bufs=2) as outp,
            tc.tile_pool(name="cst", bufs=1) as cst,
            tc.tile_pool(name="ps", bufs=2, space=MemorySpace.PSUM) as psp,
        ):
            mt = cst.tile([128, NMASK, S], bf16)
            nc.sync.dma_start(mt[:], mk.rearrange("g k m -> k g m"))

            zt = cst.tile([128, FILLW], bf16, name="zt")
            nc.vector.memset(zt[:], 0.0)

            x1t = x1p.tile([128, BL, HW], bf16)
            nc.sync.dma_start(x1t[:, 0, :], x1[:, 0, :])

            def filler(pst, j, n=NFILL):
                # zero-data accumulates with whatever weights are resident:
                # keeps the PE busy through the per-pair DMA gap so the power
                # manager holds full clock; adds 0.0 to live PSUM
                for k in range(n):
                    inst = nc.tensor.matmul(
                        pst[(j + k) % NQ][:, :],
                        mt[:, ZW, :],
                        zt[:, :],
                        start=False,
                        stop=False,
                        skip_group_check=True,
                    )
                    inst.ins.ldweights = False
                    filler_names.add(inst.ins.name)

            for b in range(BL):
                last = b == BL - 1

                pst = [
                    psp.tile([S, QW], f32, name=f"ps{q}", tag=f"ps{q}")
                    for q in range(NQ)
                ]

                if not last:
                    # leftover support 24 first: its sqrt/store tail then
                    # overlaps the next batch's stream
                    x2l = lop.tile([64, HW], bf16, tag="lo")
                    nc.sync.dma_start(x2l[:], x2[b, S - 1])

                # per-pair loads; batch b+1's x1 slice rides after pair 0
                pairs = []
                for j in range(NPAIR):
                    x2t = x2p.tile([128, HW], bf16, tag="x2t")
                    nc.sync.dma_start(
                        x2t[:], x2[b, 2 * j : 2 * j + 2].rearrange("si c p -> (si c) p")
                    )
                    pairs.append(x2t)
                    if j == 0 and b + 1 < BL:
                        nc.sync.dma_start(x1t[:, b + 1, :], x1[:, b + 1, :])
                if last:
                    # leftover is the kernel tail: quarter-sliced, loaded last
                    x2l = lop.tile([64, HW], bf16, tag="lo")
                    for q in range(NQ):
                        nc.sync.dma_start(
                            x2l[:, q * QW : (q + 1) * QW],
                            x2[b, S - 1][:, q * QW : (q + 1) * QW],
                        )

                if not last:
                    nc.vector.tensor_tensor(x2l[:], x2l[:], x1t[0:64, b, :], sub)
                    sql = sqlp.tile([64, HW], bf16, name="sql", tag="sql")
                    nc.scalar.activation(sql[:], x2l[:], Square)
                    lo_loader = None
                    for q in range(NQ):
                        inst = mm(
                            pst[q][:, :],
                            mt[0:64, LO, :],
                            sql[:, q * QW : (q + 1) * QW],
                            start=True,
                            stop=False,
                            loader=lo_loader,
                        )
                        if lo_loader is None:
                            lo_loader = inst
                    filler(pst, 0, 2)

                for j in range(NPAIR):
                    x2t = pairs[j]
                    nc.vector.tensor_tensor(x2t[:], x2t[:], x1t[:, b, :], sub)
                    sq = sqp.tile([128, HW], bf16, tag="sq")
                    if SQ_ENG[j] == "A":
                        nc.scalar.activation(sq[:], x2t[:], Square)
                    else:
                        nc.vector.tensor_tensor(sq[:], x2t[:], x2t[:], mul)
                    loader = None
                    for q in range(NQ):
                        inst = mm(
                            pst[q][:, :],
                            mt[:, j, :],
                            sq[:, q * QW : (q + 1) * QW],
                            start=(last and j == 0),
                            stop=(not last and j == NPAIR - 1),
                            loader=loader,
                            skip=(not last and j == NPAIR - 1),
                        )
                        if loader is None:
                            loader = inst
                    filler(pst, j)

                if not last:
                    ot = outp.tile([S, HW], bf16, name="ot", tag="ot")
                    for q in range(NQ):
                        nc.scalar.activation(
                            ot[:, q * QW : (q + 1) * QW], pst[q][:], Sqrt
                        )
                    nc.scalar.dma_start(out[b].rearrange("(s p) -> s p", s=S), ot[:])
                else:
                    # tail: leftover quarters stream in as the final DMAs;
                    # each quarter's chain fires on its own 56KB completion
                    ot = outp.tile([S, HW], bf16, name="ot", tag="ot")
                    sql = sqlp.tile([64, HW], bf16, name="sql", tag="sql")
                    lo_loader = None
                    for q in range(NQ):
                        qs = slice(q * QW, (q + 1) * QW)
                        nc.vector.tensor_tensor(
                            x2l[:, qs], x2l[:, qs], x1t[0:64, b, qs], sub
                        )
                        nc.scalar.activation(sql[:, qs], x2l[:, qs], Square)
                        inst = mm(
                            pst[q][:, :],
                            mt[0:64, LO, :],
                            sql[:, qs],
                            start=False,
                            stop=True,
                            loader=lo_loader,
                        )
                        if lo_loader is None:
                            lo_loader = inst
                        nc.scalar.activation(ot[:, qs], pst[q][:], Sqrt)
                        nc.scalar.dma_start(
                            out[b].rearrange("(s p) -> s p", s=S)[:, qs], ot[:, qs]
                        )

    try:
        nc.finalize()
    finally:
        bacc.get_activation_tables = _orig_tables
    if ELIDE_LDW:
        _verify_ldw_order(nc, elide_owner, filler_names)
    return nc


def _verify_ldw_order(nc, elide_owner, filler_names):
    """The 4 quarter-matmuls of a pair share one weight load. Walk the final
    (post-Tile-scheduling) program order and assert no other weight-loading
    matmult lands between a loader and its elided dependents."""
    last_loader = None
    for blk in nc.m.functions[0].blocks:
        for inst in blk.instructions:
            if type(inst).__name__ != "InstMatmult":
                continue
            name = inst.name
            if name in filler_names:
                continue  # zero moving data: any weights give 0
            if name in elide_owner:
                if last_loader != elide_owner[name]:
                    raise RuntimeError(
                        f"ldweights elision unsafe: {name} expects weights of "
                        f"{elide_owner[name]} but last loader is {last_loader}"
                    )
            else:
                last_loader = name


def get_nc():
    if "nc" not in _cache:
        _cache["nc"] = _build_nc()
    return _cache["nc"]


def make_mask() -> np.ndarray:
    # mask[j, k, m] = 1 iff partition k of pair-tile j feeds output support m.
    # Pair j < 12 covers supports (2j, 2j+1): k < 64 -> 2j, k >= 64 -> 2j+1.
    # Slot 12 is the leftover single support 24 on partitions 0..63.
    # Slot 13 is all zeros: weights for the PE keep-warm filler matmuls.
    import ml_dtypes

    mask = np.zeros((NMASK, 128, S), dtype=ml_dtypes.bfloat16)
    for j in range(NPAIR):
        mask[j, 0:64, 2 * j] = 1.0
        mask[j, 64:128, 2 * j + 1] = 1.0
    mask[LO, 0:64, S - 1] = 1.0
    return mask


def make_in_maps(x1: np.ndarray, x2: np.ndarray) -> list[dict]:
    import ml_dtypes

    bf16 = ml_dtypes.bfloat16
    x1 = np.asarray(x1, dtype=np.float32).reshape(B, C, HW)
    x2 = np.asarray(x2, dtype=np.float32).reshape(B, S, C, HW)
    mask = make_mask()
    maps = []
    for i in range(NCORES):
        sl = slice(i * BL, (i + 1) * BL)
        # x1 staged bf16, channel-major, duplicated onto both partition
        # halves so it aligns with the (si c) pair layout
        x1c = np.ascontiguousarray(x1[sl].transpose(1, 0, 2)).astype(bf16)
        x1d = np.ascontiguousarray(np.concatenate([x1c, x1c], axis=0))
        x2c = np.ascontiguousarray(x2[sl]).astype(bf16)
        maps.append({"x1": x1d, "x2": x2c, "mask": mask})
    return maps


def gather_out(results: list[dict]) -> np.ndarray:
    return np.concatenate([np.asarray(r["out"]) for r in results], axis=0).astype(
        np.float32
    )


def kernel(x1, x2) -> np.ndarray:
    from concourse.bass_utils import run_bass_kernel_spmd

    nc = get_nc()
    in_maps = make_in_maps(x1, x2)
    res = run_bass_kernel_spmd(nc, in_maps, list(range(NCORES)))
    return gather_out(res.results)
